# revision 7
# baseline (speedup 1.0000x reference)
"""Multi-head attention kernel for 8 Trainium2 NeuronCores.

Problem: B=4, S=2048, D=1024, H=16, Dh=64 MHA with key-side boolean mask.

Sharding: core c handles (batch b = c//2, head-half g = c%2, 8 heads each).
QKV are column-parallel, the output projection is row-parallel (Megatron
style); the host sums the two partial output projections per batch and adds
the output bias.

Host-side preprocessing (pure data marshalling, exact):
  - All inputs are pre-tiled into DMA-native layouts (partition-major,
    contiguous per partition) so each dma_start lowers to large linear
    descriptors instead of thousands of 2KB strided reads.
  - x is transposed per batch (the PE contracts over the partition dim, so
    x^T is required for every projection).
  - Keys with mask=False contribute exactly zero after softmax, so the host
    gathers only the unmasked keys (padded to a multiple of 384 with zero
    rows whose exp-bias is -1e30 => exp == 0 exactly). Exact, and cuts
    score/exp/attn-V work roughly in half.

On-core dataflow (all matmuls in float32r):
  xT --(Wk,Wv)--> KT[f,k], V[k,f] (+biases); KT rows 0:64 even head,
                  rows 64:128 odd head of each pair
  xT --(Wq)--> QT[f,q]
  scores^T[k,q]: per head PAIR, two row-tiled K=64 matmuls run
                 concurrently on disjoint PE row-groups (~the cost of one)
  E = exp(scores*0.125 + maskbias[k])      (one ScalarE pass, mask fused)
  out_aug[65,q] = [V_h | ones]^T x E       (row 64 = softmax denominator)
  rX[1,q] = 1/den (DVE exact reciprocal, off the PE critical path)
  rbX[64,q] = SBUF->SBUF DMA broadcast of rX (free-dim stride-0 read)
  attnT[f,q] = out_aug[0:64] * rbX         (DVE, one PSUM operand)
  out[s,D] = attnT^T x Wo                  (partial; host adds pair + bo)
"""

import os
import numpy as np

os.environ.setdefault("MYCRO_LOCAL_CACHE", "1")

D_MODEL = 1024
N_HEADS = 16
D_HEAD = 64
BATCH = 4
SEQ = 2048
N_CORES = 8
FH = 512          # features per core (8 heads x 64)
HPC = 8           # heads per core
NEG = -1.0e30     # additive bias for padded/masked keys; exp -> 0 exactly

_COMPILED = {}    # k_pad -> nc
last_results = None  # BassKernelResults of the most recent run (for test.py)


def _build(k_pad):
    """Emit + compile the per-core bass kernel for a given padded key count."""
    import concourse.bacc as bacc
    import concourse.bass as bass
    import concourse.tile as tile
    from concourse import mybir

    f32 = mybir.dt.float32
    f32r = mybir.dt.float32r
    KT_N = k_pad // 128                     # number of 128-key tiles
    KC = 512 if k_pad % 512 == 0 else 384   # key-side chunk (fp32r needs N>=256)
    assert k_pad % KC == 0 and KC % 128 == 0
    NKC = k_pad // KC

    nc = bacc.Bacc("TRN2", target_bir_lowering=False, debug=False,
                   num_devices=N_CORES)

    # all pre-tiled on host into DMA-native layouts
    dxq = nc.dram_tensor("xq", [4, 128, 8, 512], f32r, kind="ExternalInput")
    dxk = nc.dram_tensor("xk", [NKC, 128, 8, KC], f32r, kind="ExternalInput")
    dWq = nc.dram_tensor("Wq", [128, 8, FH], f32r, kind="ExternalInput")
    dWk = nc.dram_tensor("Wk", [128, 8, FH], f32r, kind="ExternalInput")
    dWv = nc.dram_tensor("Wv", [128, 8, HPC * 65], f32r, kind="ExternalInput")
    dWo = nc.dram_tensor("Wo", [128, 4, D_MODEL], f32r, kind="ExternalInput")
    dbc = nc.dram_tensor("bcst", [128, 8 + KT_N], f32, kind="ExternalInput")
    dbv = nc.dram_tensor("bv", [HPC * 65], f32r, kind="ExternalInput")
    dcst = nc.dram_tensor("consts", [256], f32r, kind="ExternalInput")  # ones
    dout = nc.dram_tensor("out", [SEQ, D_MODEL], f32, kind="ExternalOutput")

    EXP = mybir.ActivationFunctionType.Exp
    IDn = mybir.ActivationFunctionType.Identity

    with tile.TileContext(nc) as tc:
        with tc.tile_pool(name="persist", bufs=1) as pers:
            # ---- constants in SBUF ----
            bc = pers.tile([128, 8 + KT_N], f32, tag="bcst")
            nc.sync.dma_start(out=bc, in_=dbc.ap())
            bq = bc[:, 0:4]
            bk = bc[:, 4:8]
            mb = bc[:, 8:8 + KT_N]
            bv_row = pers.tile([1, HPC * 65], f32r, tag="bvr")
            nc.sync.dma_start(out=bv_row, in_=dbv.ap()[None, :])
            ones_t = pers.tile([1, 128], f32r, tag="ones")
            nc.sync.dma_start(out=ones_t, in_=dcst.ap()[None, 0:128])
            ones128 = ones_t[:, :]

            # ---- persistent activations ----
            QT = pers.tile([128, 4, SEQ], f32r, tag="QT")        # [f, q]
            # KT rows 0:64 = even head of pair, rows 64:128 = odd head.
            KT = pers.tile([128, 4, k_pad], f32r, tag="KT")
            Vau = pers.tile([128, KT_N, HPC, 65], f32r, tag="Vau")

            # ================= projections =================
            wtq_cm = tc.tile_pool(name="wtq", bufs=1)
            wtq = wtq_cm.__enter__()
            wq = wtq.tile([128, 8, FH], f32r, tag="wq")
            nc.sync.dma_start(out=wq, in_=dWq.ap())
            ppool_cm = tc.tile_pool(name="pp", bufs=4, space="PSUM")
            ppool = ppool_cm.__enter__()

            # ----- K side (KT, V) -----
            with tc.tile_pool(name="wtk", bufs=1) as wtk, \
                 tc.tile_pool(name="xk", bufs=2) as xkp:
                pk = ppool
                wk = wtk.tile([128, 8, FH], f32r, tag="wk")
                nc.sync.dma_start(out=wk, in_=dWk.ap())
                wv = wtk.tile([128, 8, HPC * 65], f32r, tag="wv")
                nc.sync.dma_start(out=wv, in_=dWv.ap())
                for kc in range(NKC):
                    xk_t = xkp.tile([128, 8, KC], f32r, tag="xk")
                    nc.sync.dma_start(out=xk_t, in_=dxk.ap()[kc])
                    for ft in range(4):
                        ps = pk.tile([128, KC], f32, tag="pk")
                        for dt in range(8):
                            nc.tensor.matmul(
                                ps,
                                lhsT=wk[:, dt, ft * 128:(ft + 1) * 128],
                                rhs=xk_t[:, dt, :],
                                start=(dt == 0), stop=(dt == 7))
                        ks = slice(kc * KC, (kc + 1) * KC)
                        nc.scalar.activation(KT[:, ft, ks], ps, IDn,
                                             bias=bk[:, ft:ft + 1])
                    for kb in range(KC // 128):
                        kg = kc * (KC // 128) + kb
                        ps = pk.tile([128, HPC * 65], f32, tag="pk")
                        for dt in range(8):
                            nc.tensor.matmul(
                                ps[:, 0:512],
                                lhsT=xk_t[:, dt, kb * 128:(kb + 1) * 128],
                                rhs=wv[:, dt, 0:512],
                                start=(dt == 0), stop=False)
                            nc.tensor.matmul(
                                ps[:, 512:520],
                                lhsT=xk_t[:, dt, kb * 128:(kb + 1) * 128],
                                rhs=wv[:, dt, 512:520],
                                start=(dt == 0), stop=False)
                        nc.tensor.matmul(ps[:, 0:512], lhsT=ones128,
                                         rhs=bv_row[:, 0:512],
                                         start=False, stop=True)
                        nc.tensor.matmul(ps[:, 512:520], lhsT=ones128,
                                         rhs=bv_row[:, 512:520],
                                         start=False, stop=True)
                        nc.scalar.copy(Vau[:, kg, :, :], ps)

            # ----- Q side (QT) -----
            with tc.tile_pool(name="xq", bufs=2) as xqp:
                pq = ppool
                for qc in range(4):
                    xq_t = xqp.tile([128, 8, 512], f32r, tag="xq")
                    nc.sync.dma_start(out=xq_t, in_=dxq.ap()[qc])
                    for ft in range(4):
                        ps = pq.tile([128, 512], f32, tag="pk")
                        for dt in range(8):
                            nc.tensor.matmul(
                                ps,
                                lhsT=wq[:, dt, ft * 128:(ft + 1) * 128],
                                rhs=xq_t[:, dt, :],
                                start=(dt == 0), stop=(dt == 7))
                        nc.scalar.activation(QT[:, ft, qc * 512:(qc + 1) * 512],
                                             ps, IDn, bias=bq[:, ft:ft + 1])

            ppool_cm.__exit__(None, None, None)
            wtq_cm.__exit__(None, None, None)

            # ================= attention core =================
            att2_cm = tc.tile_pool(name="att2", bufs=1)
            att2 = att2_cm.__enter__()
            attnT = att2.tile([128, 4, SEQ], f32r, tag="attnT")  # [f, q]
            wo = att2.tile([128, 4, D_MODEL], f32r, tag="wo")
            nc.sync.dma_start(out=wo, in_=dWo.ap())
            with tc.tile_pool(name="et", bufs=4) as etp, \
                 tc.tile_pool(name="dn", bufs=2) as dnp, \
                 tc.tile_pool(name="rb", bufs=4) as rbp, \
                 tc.tile_pool(name="sp", bufs=2, space="PSUM") as sp, \
                 tc.tile_pool(name="av", bufs=2, space="PSUM") as avp:
                for qh in range(2):         # query half (1024 queries)
                    q0 = qh * 1024
                    # all 8 softmax denominators of this query half
                    dden = dnp.tile([8, 1024], f32, tag="dden")
                    for t in range(4):      # head pair (heads 2t, 2t+1)
                        avA = avp.tile([65, 1024], f32, tag="av")
                        avB = avp.tile([65, 1024], f32, tag="av")
                        for kt in range(KT_N):
                            kts = slice(kt * 128, (kt + 1) * 128)
                            sA = sp.tile([128, 1024], f32, tag="s")
                            sB = sp.tile([128, 1024], f32, tag="s")
                            for h in range(2):
                                c0, c1 = q0 + h * 512, q0 + (h + 1) * 512
                                cs = slice(h * 512, (h + 1) * 512)
                                # row-tiled concurrent K=64 pair
                                nc.tensor.matmul(
                                    sA[:, cs],
                                    lhsT=KT[0:64, t, kts],
                                    rhs=QT[0:64, t, c0:c1],
                                    start=True, stop=True)
                                nc.tensor.matmul(
                                    sB[:, cs],
                                    lhsT=KT[64:128, t, kts],
                                    rhs=QT[64:128, t, c0:c1],
                                    start=True, stop=True)
                            eA = etp.tile([128, 1024], f32r, tag="et")
                            nc.scalar.activation(eA, sA, EXP,
                                                 bias=mb[:, kt:kt + 1], scale=0.125)
                            eB = etp.tile([128, 1024], f32r, tag="et")
                            nc.scalar.activation(eB, sB, EXP,
                                                 bias=mb[:, kt:kt + 1], scale=0.125)
                            for h in range(2):
                                cs = slice(h * 512, (h + 1) * 512)
                                nc.tensor.matmul(
                                    avA[:, cs], lhsT=Vau[:, kt, 2 * t, :],
                                    rhs=eA[:, cs],
                                    start=(kt == 0), stop=(kt == KT_N - 1))
                                nc.tensor.matmul(
                                    avB[:, cs], lhsT=Vau[:, kt, 2 * t + 1, :],
                                    rhs=eB[:, cs],
                                    start=(kt == 0), stop=(kt == KT_N - 1))
                        # drain PSUM quickly: unnormalized attn rows into
                        # attnT; denominator rows stage at partition 0 (DVE
                        # base-partition rule) then DMA-hop into dden row.
                        for hh, (av_t, prow) in enumerate(((avA, 0), (avB, 64))):
                            nc.vector.tensor_copy(
                                attnT[prow:prow + 64, t, q0:q0 + 1024],
                                av_t[0:64, :])
                            dstg = rbp.tile([1, 1024], f32, tag="dstg")
                            nc.vector.tensor_copy(dstg, av_t[64:65, :])
                            nc.sync.dma_start(
                                out=dden[2 * t + hh:2 * t + hh + 1, :], in_=dstg)
                    # one batched reciprocal for the whole query half, then
                    # broadcast each row via SBUF->SBUF DMA and scale in place.
                    drec = dnp.tile([8, 1024], f32, tag="drec")
                    nc.vector.reciprocal(drec, dden)
                    for t in range(4):
                        rb_t = rbp.tile([128, 1024], f32, tag="rb")
                        for hh in range(2):
                            row = drec[2 * t + hh:2 * t + hh + 1, :]
                            bsrc = bass.AP(tensor=row.tensor, offset=row.offset,
                                           ap=[[row.ap[0][0], 1], [0, 64],
                                               [1, 1024]])
                            nc.sync.dma_start(out=rb_t[hh * 64:(hh + 1) * 64, :],
                                              in_=bsrc)
                        for hh in range(2):
                            prow = hh * 64
                            sl = attnT[prow:prow + 64, t, q0:q0 + 1024]
                            nc.vector.tensor_mul(sl, sl,
                                                 rb_t[prow:prow + 64, :])

            # ================= output projection (partial) =================
            with tc.tile_pool(name="op", bufs=2, space="PSUM") as opp, \
                 tc.tile_pool(name="ot", bufs=3) as otp:
                for st in range(16):
                    ps = opp.tile([128, D_MODEL], f32, tag="op")
                    for ft in range(4):
                        for dh in range(2):
                            nc.tensor.matmul(
                                ps[:, dh * 512:(dh + 1) * 512],
                                lhsT=attnT[:, ft, st * 128:(st + 1) * 128],
                                rhs=wo[:, ft, dh * 512:(dh + 1) * 512],
                                start=(ft == 0), stop=(ft == 3))
                    ot = otp.tile([128, D_MODEL], f32, tag="ot")
                    nc.vector.tensor_copy(ot, ps)
                    nc.sync.dma_start(out=dout.ap()[st * 128:(st + 1) * 128, :], in_=ot)
            att2_cm.__exit__(None, None, None)

    nc.compile()
    return nc


def _get_compiled(k_pad):
    if k_pad not in _COMPILED:
        _COMPILED[k_pad] = _build(k_pad)
    return _COMPILED[k_pad]


def _tile_pf(a, p=128):
    """[P*t, f...] -> contiguous [p, t, f...] partition-major tiling."""
    t = a.shape[0] // p
    return np.ascontiguousarray(
        a.reshape(t, p, *a.shape[1:]).swapaxes(0, 1))


def _prep_core_inputs(x, attention_mask, Wq, bq, Wk, bk, Wv, bv, Wo):
    """Host-side shard prep. Returns (in_maps, k_pad)."""
    x = np.asarray(x, np.float32)
    mask = np.asarray(attention_mask, bool)
    idxs = [np.nonzero(mask[b])[0] for b in range(BATCH)]
    ke_max = max(1, max(len(i) for i in idxs))
    k_pad = 384 * ((ke_max + 383) // 384)
    if k_pad > SEQ:
        k_pad = SEQ
    KC = 512 if k_pad % 512 == 0 else 384
    NKC = k_pad // KC
    KT_N = k_pad // 128

    consts = np.zeros(256, np.float32)
    consts[0:128] = 1.0

    in_maps = []
    for b in range(BATCH):
        xT = x[b].T                                  # [D, S] view
        # xq: [qc, p, dt, 512]
        xq = np.ascontiguousarray(
            xT.reshape(8, 128, 4, 512).transpose(2, 1, 0, 3))
        idx = idxs[b]
        ke = len(idx)
        if ke > k_pad:
            idx = idx[:k_pad]
            ke = k_pad
        xkT = np.zeros((D_MODEL, k_pad), np.float32)
        xkT[:, :ke] = x[b][idx].T
        # xk: [kc, p, dt, KC]
        xk = np.ascontiguousarray(
            xkT.reshape(8, 128, NKC, KC).transpose(2, 1, 0, 3))
        maskb = np.zeros(k_pad, np.float32)
        maskb[ke:] = NEG
        mb_t = _tile_pf(maskb)                       # [128, KT_N]
        KT_N = k_pad // 128
        for g in range(2):
            fs = slice(g * FH, (g + 1) * FH)
            # Wv/bv padded with a ones column per head: the V-projection
            # matmul then produces [V_h | ones] directly (col = 0*x + 1.0).
            Wv_aug = np.zeros((D_MODEL, HPC * 65), np.float32)
            bv_aug = np.zeros(HPC * 65, np.float32)
            for h in range(HPC):
                Wv_aug[:, h * 65:h * 65 + 64] = Wv[:, g * FH + h * 64:
                                                   g * FH + (h + 1) * 64]
                bv_aug[h * 65:h * 65 + 64] = bv[g * FH + h * 64:
                                                g * FH + (h + 1) * 64]
                bv_aug[h * 65 + 64] = 1.0
            in_maps.append({
                "xq": xq,
                "xk": xk,
                "Wq": _tile_pf(np.asarray(Wq[:, fs], np.float32)),
                "Wk": _tile_pf(np.asarray(Wk[:, fs], np.float32)),
                "Wv": _tile_pf(Wv_aug),
                "Wo": _tile_pf(np.asarray(Wo[fs, :], np.float32)),
                "bcst": np.concatenate(
                    [_tile_pf(np.asarray(bq[fs], np.float32)),
                     _tile_pf(np.asarray(bk[fs], np.float32)),
                     mb_t], axis=1).astype(np.float32),
                "bv": bv_aug,
                "consts": consts,
            })
    return in_maps, k_pad


def kernel(x, attention_mask, Wq, bq, Wk, bk, Wv, bv, Wo, bo):
    global last_results
    from concourse.bass_utils import run_bass_kernel_spmd

    in_maps, k_pad = _prep_core_inputs(x, attention_mask, Wq, bq, Wk, bk, Wv, bv, Wo)
    nc = _get_compiled(k_pad)
    res = run_bass_kernel_spmd(nc, in_maps, core_ids=list(range(N_CORES)))
    last_results = res

    bo = np.asarray(bo, np.float32)
    out = np.empty((BATCH, SEQ, D_MODEL), np.float32)
    for b in range(BATCH):
        out[b] = res.results[2 * b]["out"] + res.results[2 * b + 1]["out"] + bo
    return out


# revision 11
# speedup vs baseline: 1.1682x; 1.1682x over previous
"""Multi-head attention kernel for 8 Trainium2 NeuronCores.

Problem: B=4, S=2048, D=1024, H=16, Dh=64 MHA with key-side boolean mask.

Sharding: core c handles (batch b = c//2, head-half g = c%2, 8 heads each).
QKV are column-parallel, the output projection is row-parallel (Megatron
style); the host sums the two partial output projections per batch and adds
the output bias.

Host-side preprocessing (pure data marshalling, exact):
  - All inputs are pre-tiled into DMA-native layouts (partition-major,
    contiguous per partition) so each dma_start lowers to large linear
    descriptors instead of thousands of 2KB strided reads.
  - x is transposed per batch (the PE contracts over the partition dim, so
    x^T is required for every projection).
  - Keys with mask=False contribute exactly zero after softmax, so the host
    gathers only the unmasked keys (padded to a multiple of 384 with zero
    rows whose exp-bias is -1e30 => exp == 0 exactly). Exact, and cuts
    score/exp/attn-V work roughly in half.

On-core dataflow (all matmuls in float32r):
  xT --(Wk,Wv)--> KT[f,k], V[k,f] (+biases); KT rows 0:64 even head,
                  rows 64:128 odd head of each pair
  xT --(Wq)--> QT[f,q]
  scores^T[k,q]: per head PAIR, two row-tiled K=64 matmuls run
                 concurrently on disjoint PE row-groups (~the cost of one)
  E = exp(scores*0.125 + maskbias[k])      (one ScalarE pass, mask fused)
  out_aug[65,q] = [V_h | ones]^T x E       (row 64 = softmax denominator)
  rX[1,q] = 1/den (DVE exact reciprocal, off the PE critical path)
  rbX[64,q] = SBUF->SBUF DMA broadcast of rX (free-dim stride-0 read)
  attnT[f,q] = out_aug[0:64] * rbX         (DVE, one PSUM operand)
  out[s,D] = attnT^T x Wo                  (partial; host adds pair + bo)
"""

import os
import numpy as np

os.environ.setdefault("MYCRO_LOCAL_CACHE", "1")

D_MODEL = 1024
N_HEADS = 16
D_HEAD = 64
BATCH = 4
SEQ = 2048
N_CORES = 8
FH = 512          # features per core (8 heads x 64)
HPC = 8           # heads per core
NEG = -1.0e30     # additive bias for padded/masked keys; exp -> 0 exactly

_COMPILED = {}    # k_pad -> nc
last_results = None  # BassKernelResults of the most recent run (for test.py)


def _build(k_pad):
    """Emit + compile the per-core bass kernel for a given padded key count."""
    import concourse.bacc as bacc
    import concourse.bass as bass
    import concourse.tile as tile
    from concourse import library_config, mybir

    f32 = mybir.dt.float32
    f32r = mybir.dt.float32r
    KT_N = k_pad // 128                     # number of 128-key tiles
    KC = 512 if k_pad % 512 == 0 else 384   # key-side chunk (fp32r needs N>=256)
    assert k_pad % KC == 0 and KC % 128 == 0
    NKC = k_pad // KC

    nc = bacc.Bacc("TRN2", target_bir_lowering=False, debug=False,
                   num_devices=N_CORES)

    # all pre-tiled on host into DMA-native layouts
    dxq = nc.dram_tensor("xq", [4, 128, 8, 512], f32r, kind="ExternalInput")
    dxk = nc.dram_tensor("xk", [NKC, 128, 8, KC], f32r, kind="ExternalInput")
    dWq = nc.dram_tensor("Wq", [128, 8, FH], f32r, kind="ExternalInput")
    dWk = nc.dram_tensor("Wk", [128, 8, FH], f32r, kind="ExternalInput")
    dWv = nc.dram_tensor("Wv", [128, 8, HPC * 65], f32r, kind="ExternalInput")
    dWo = nc.dram_tensor("Wo", [128, 4, D_MODEL], f32r, kind="ExternalInput")
    dbc = nc.dram_tensor("bcst", [128, 8 + KT_N], f32, kind="ExternalInput")
    dbv = nc.dram_tensor("bv", [HPC * 65], f32r, kind="ExternalInput")
    dcst = nc.dram_tensor("consts", [256], f32r, kind="ExternalInput")  # ones
    dout = nc.dram_tensor("out", [SEQ, D_MODEL], f32, kind="ExternalOutput")

    EXP = mybir.ActivationFunctionType.Exp
    IDn = mybir.ActivationFunctionType.Identity

    with tile.TileContext(nc) as tc:
        nc.gpsimd.load_library(library_config.attn)
        with tc.tile_pool(name="persist", bufs=1) as pers:
            # ---- constants in SBUF ----
            bc = pers.tile([128, 8 + KT_N], f32, tag="bcst")
            nc.sync.dma_start(out=bc, in_=dbc.ap())
            bq = bc[:, 0:4]
            bk = bc[:, 4:8]
            mb = bc[:, 8:8 + KT_N]
            bv_row = pers.tile([1, HPC * 65], f32r, tag="bvr")
            nc.sync.dma_start(out=bv_row, in_=dbv.ap()[None, :])
            ones_t = pers.tile([1, 128], f32r, tag="ones")
            nc.sync.dma_start(out=ones_t, in_=dcst.ap()[None, 0:128])
            ones128 = ones_t[:, :]

            # ---- persistent activations ----
            QT = pers.tile([128, 4, SEQ], f32r, tag="QT")        # [f, q]
            # KT rows 0:64 = even head of pair, rows 64:128 = odd head.
            KT = pers.tile([128, 4, k_pad], f32r, tag="KT")
            Vau = pers.tile([128, KT_N, HPC, 65], f32r, tag="Vau")

            # ================= projections =================
            wtq_cm = tc.tile_pool(name="wtq", bufs=1)
            wtq = wtq_cm.__enter__()
            wq = wtq.tile([128, 8, FH], f32r, tag="wq")
            nc.sync.dma_start(out=wq, in_=dWq.ap())
            ppool_cm = tc.tile_pool(name="pp", bufs=4, space="PSUM")
            ppool = ppool_cm.__enter__()

            # ----- K side (KT, V) -----
            with tc.tile_pool(name="wtk", bufs=1) as wtk, \
                 tc.tile_pool(name="xk", bufs=2) as xkp:
                pk = ppool
                wk = wtk.tile([128, 8, FH], f32r, tag="wk")
                nc.sync.dma_start(out=wk, in_=dWk.ap())
                wv = wtk.tile([128, 8, HPC * 65], f32r, tag="wv")
                nc.sync.dma_start(out=wv, in_=dWv.ap())
                for kc in range(NKC):
                    xk_t = xkp.tile([128, 8, KC], f32r, tag="xk")
                    nc.sync.dma_start(out=xk_t, in_=dxk.ap()[kc])
                    for ft in range(4):
                        ps = pk.tile([128, KC], f32, tag="pk")
                        for dt in range(8):
                            nc.tensor.matmul(
                                ps,
                                lhsT=wk[:, dt, ft * 128:(ft + 1) * 128],
                                rhs=xk_t[:, dt, :],
                                start=(dt == 0), stop=(dt == 7))
                        ks = slice(kc * KC, (kc + 1) * KC)
                        nc.scalar.activation(KT[:, ft, ks], ps, IDn,
                                             bias=bk[:, ft:ft + 1])
                    for kb in range(KC // 128):
                        kg = kc * (KC // 128) + kb
                        ps = pk.tile([128, HPC * 65], f32, tag="pk")
                        for dt in range(8):
                            nc.tensor.matmul(
                                ps[:, 0:512],
                                lhsT=xk_t[:, dt, kb * 128:(kb + 1) * 128],
                                rhs=wv[:, dt, 0:512],
                                start=(dt == 0), stop=False)
                            nc.tensor.matmul(
                                ps[:, 512:520],
                                lhsT=xk_t[:, dt, kb * 128:(kb + 1) * 128],
                                rhs=wv[:, dt, 512:520],
                                start=(dt == 0), stop=False)
                        nc.tensor.matmul(ps[:, 0:512], lhsT=ones128,
                                         rhs=bv_row[:, 0:512],
                                         start=False, stop=True)
                        nc.tensor.matmul(ps[:, 512:520], lhsT=ones128,
                                         rhs=bv_row[:, 512:520],
                                         start=False, stop=True)
                        nc.scalar.copy(Vau[:, kg, :, :], ps)

            # ----- Q side (QT) -----
            with tc.tile_pool(name="xq", bufs=2) as xqp:
                pq = ppool
                for qc in range(4):
                    xq_t = xqp.tile([128, 8, 512], f32r, tag="xq")
                    nc.sync.dma_start(out=xq_t, in_=dxq.ap()[qc])
                    for ft in range(4):
                        ps = pq.tile([128, 512], f32, tag="pk")
                        for dt in range(8):
                            nc.tensor.matmul(
                                ps,
                                lhsT=wq[:, dt, ft * 128:(ft + 1) * 128],
                                rhs=xq_t[:, dt, :],
                                start=(dt == 0), stop=(dt == 7))
                        nc.scalar.activation(QT[:, ft, qc * 512:(qc + 1) * 512],
                                             ps, IDn, bias=bq[:, ft:ft + 1])

            ppool_cm.__exit__(None, None, None)
            wtq_cm.__exit__(None, None, None)

            # ================= attention core =================
            att2_cm = tc.tile_pool(name="att2", bufs=1)
            att2 = att2_cm.__enter__()
            attnT = att2.tile([128, 4, SEQ], f32r, tag="attnT")  # [f, q]
            wo = att2.tile([128, 4, D_MODEL], f32r, tag="wo")
            nc.sync.dma_start(out=wo, in_=dWo.ap())
            with tc.tile_pool(name="et", bufs=4) as etp, \
                 tc.tile_pool(name="dn", bufs=2) as dnp, \
                 tc.tile_pool(name="rb", bufs=2) as rbp, \
                 tc.tile_pool(name="sp", bufs=2, space="PSUM") as sp, \
                 tc.tile_pool(name="av", bufs=2, space="PSUM") as avp:
                for qh in range(2):         # query half (1024 queries)
                    q0 = qh * 1024
                    # all 8 softmax denominators of this query half
                    dden = dnp.tile([8, 1024], f32, tag="dden")
                    for t in range(4):      # head pair (heads 2t, 2t+1)
                        avA = avp.tile([65, 1024], f32, tag="av")
                        avB = avp.tile([65, 1024], f32, tag="av")
                        for kt in range(KT_N):
                            kts = slice(kt * 128, (kt + 1) * 128)
                            sA = sp.tile([128, 1024], f32, tag="s")
                            sB = sp.tile([128, 1024], f32, tag="s")
                            for h in range(2):
                                c0, c1 = q0 + h * 512, q0 + (h + 1) * 512
                                cs = slice(h * 512, (h + 1) * 512)
                                # row-tiled concurrent K=64 pair
                                nc.tensor.matmul(
                                    sA[:, cs],
                                    lhsT=KT[0:64, t, kts],
                                    rhs=QT[0:64, t, c0:c1],
                                    start=True, stop=True)
                                nc.tensor.matmul(
                                    sB[:, cs],
                                    lhsT=KT[64:128, t, kts],
                                    rhs=QT[64:128, t, c0:c1],
                                    start=True, stop=True)
                            eA = etp.tile([128, 1024], f32r, tag="et")
                            nc.scalar.activation(eA, sA, EXP,
                                                 bias=mb[:, kt:kt + 1], scale=0.125)
                            eB = etp.tile([128, 1024], f32r, tag="et")
                            nc.scalar.activation(eB, sB, EXP,
                                                 bias=mb[:, kt:kt + 1], scale=0.125)
                            for h in range(2):
                                cs = slice(h * 512, (h + 1) * 512)
                                nc.tensor.matmul(
                                    avA[:, cs], lhsT=Vau[:, kt, 2 * t, :],
                                    rhs=eA[:, cs],
                                    start=(kt == 0), stop=(kt == KT_N - 1))
                                nc.tensor.matmul(
                                    avB[:, cs], lhsT=Vau[:, kt, 2 * t + 1, :],
                                    rhs=eB[:, cs],
                                    start=(kt == 0), stop=(kt == KT_N - 1))
                        # drain PSUM quickly: unnormalized attn rows into
                        # attnT; denominator rows stage at partition 0 (DVE
                        # base-partition rule) then DMA-hop into dden row.
                        for hh, (av_t, prow) in enumerate(((avA, 0), (avB, 64))):
                            nc.vector.tensor_copy(
                                attnT[prow:prow + 64, t, q0:q0 + 1024],
                                av_t[0:64, :])
                            dstg = rbp.tile([1, 1024], f32, tag="dstg")
                            nc.vector.tensor_copy(dstg, av_t[64:65, :])
                            nc.sync.dma_start(
                                out=dden[2 * t + hh:2 * t + hh + 1, :], in_=dstg)
                    # one batched reciprocal for the whole query half, then
                    # broadcast each row via SBUF->SBUF DMA and scale in place.
                    drec = dnp.tile([8, 1024], f32, tag="drec")
                    nc.vector.reciprocal(drec, dden)
                    for t in range(4):
                        rb_t = rbp.tile([128, 1024], f32, tag="rb")
                        for hh in range(2):
                            rsg = rbp.tile([1, 1024], f32, tag="rsg")
                            nc.sync.dma_start(
                                out=rsg,
                                in_=drec[2 * t + hh:2 * t + hh + 1, :])
                            if hh == 0:
                                nc.gpsimd.partition_broadcast(
                                    rb_t[0:64, :], rsg, channels=64)
                            else:
                                rtmp = rbp.tile([64, 1024], f32, tag="rtmp")
                                nc.gpsimd.partition_broadcast(
                                    rtmp, rsg, channels=64)
                                nc.vector.tensor_copy(rb_t[64:128, :], rtmp)
                        for hh in range(2):
                            prow = hh * 64
                            sl = attnT[prow:prow + 64, t, q0:q0 + 1024]
                            nc.vector.tensor_mul(sl, sl,
                                                 rb_t[prow:prow + 64, :])

            # ================= output projection (partial) =================
            with tc.tile_pool(name="op", bufs=2, space="PSUM") as opp, \
                 tc.tile_pool(name="ot", bufs=3) as otp:
                for st in range(16):
                    ps = opp.tile([128, D_MODEL], f32, tag="op")
                    for ft in range(4):
                        for dh in range(2):
                            nc.tensor.matmul(
                                ps[:, dh * 512:(dh + 1) * 512],
                                lhsT=attnT[:, ft, st * 128:(st + 1) * 128],
                                rhs=wo[:, ft, dh * 512:(dh + 1) * 512],
                                start=(ft == 0), stop=(ft == 3))
                    ot = otp.tile([128, D_MODEL], f32, tag="ot")
                    nc.vector.tensor_copy(ot, ps)
                    nc.sync.dma_start(out=dout.ap()[st * 128:(st + 1) * 128, :], in_=ot)
            att2_cm.__exit__(None, None, None)

    nc.compile()
    return nc


def _get_compiled(k_pad):
    if k_pad not in _COMPILED:
        _COMPILED[k_pad] = _build(k_pad)
    return _COMPILED[k_pad]


def _tile_pf(a, p=128):
    """[P*t, f...] -> contiguous [p, t, f...] partition-major tiling."""
    t = a.shape[0] // p
    return np.ascontiguousarray(
        a.reshape(t, p, *a.shape[1:]).swapaxes(0, 1))


def _prep_core_inputs(x, attention_mask, Wq, bq, Wk, bk, Wv, bv, Wo):
    """Host-side shard prep. Returns (in_maps, k_pad)."""
    x = np.asarray(x, np.float32)
    mask = np.asarray(attention_mask, bool)
    idxs = [np.nonzero(mask[b])[0] for b in range(BATCH)]
    ke_max = max(1, max(len(i) for i in idxs))
    k_pad = 384 * ((ke_max + 383) // 384)
    if k_pad > SEQ:
        k_pad = SEQ
    KC = 512 if k_pad % 512 == 0 else 384
    NKC = k_pad // KC
    KT_N = k_pad // 128

    consts = np.zeros(256, np.float32)
    consts[0:128] = 1.0

    in_maps = []
    for b in range(BATCH):
        xT = x[b].T                                  # [D, S] view
        # xq: [qc, p, dt, 512]
        xq = np.ascontiguousarray(
            xT.reshape(8, 128, 4, 512).transpose(2, 1, 0, 3))
        idx = idxs[b]
        ke = len(idx)
        if ke > k_pad:
            idx = idx[:k_pad]
            ke = k_pad
        xkT = np.zeros((D_MODEL, k_pad), np.float32)
        xkT[:, :ke] = x[b][idx].T
        # xk: [kc, p, dt, KC]
        xk = np.ascontiguousarray(
            xkT.reshape(8, 128, NKC, KC).transpose(2, 1, 0, 3))
        maskb = np.zeros(k_pad, np.float32)
        maskb[ke:] = NEG
        mb_t = _tile_pf(maskb)                       # [128, KT_N]
        KT_N = k_pad // 128
        for g in range(2):
            fs = slice(g * FH, (g + 1) * FH)
            # Wv/bv padded with a ones column per head: the V-projection
            # matmul then produces [V_h | ones] directly (col = 0*x + 1.0).
            Wv_aug = np.zeros((D_MODEL, HPC * 65), np.float32)
            bv_aug = np.zeros(HPC * 65, np.float32)
            for h in range(HPC):
                Wv_aug[:, h * 65:h * 65 + 64] = Wv[:, g * FH + h * 64:
                                                   g * FH + (h + 1) * 64]
                bv_aug[h * 65:h * 65 + 64] = bv[g * FH + h * 64:
                                                g * FH + (h + 1) * 64]
                bv_aug[h * 65 + 64] = 1.0
            in_maps.append({
                "xq": xq,
                "xk": xk,
                "Wq": _tile_pf(np.asarray(Wq[:, fs], np.float32)),
                "Wk": _tile_pf(np.asarray(Wk[:, fs], np.float32)),
                "Wv": _tile_pf(Wv_aug),
                "Wo": _tile_pf(np.asarray(Wo[fs, :], np.float32)),
                "bcst": np.concatenate(
                    [_tile_pf(np.asarray(bq[fs], np.float32)),
                     _tile_pf(np.asarray(bk[fs], np.float32)),
                     mb_t], axis=1).astype(np.float32),
                "bv": bv_aug,
                "consts": consts,
            })
    return in_maps, k_pad


def kernel(x, attention_mask, Wq, bq, Wk, bk, Wv, bv, Wo, bo):
    global last_results
    from concourse.bass_utils import run_bass_kernel_spmd

    in_maps, k_pad = _prep_core_inputs(x, attention_mask, Wq, bq, Wk, bk, Wv, bv, Wo)
    nc = _get_compiled(k_pad)
    res = run_bass_kernel_spmd(nc, in_maps, core_ids=list(range(N_CORES)))
    last_results = res

    bo = np.asarray(bo, np.float32)
    out = np.empty((BATCH, SEQ, D_MODEL), np.float32)
    for b in range(BATCH):
        out[b] = res.results[2 * b]["out"] + res.results[2 * b + 1]["out"] + bo
    return out


# revision 27
# speedup vs baseline: 1.2767x; 1.0929x over previous
"""Multi-head attention kernel for 8 Trainium2 NeuronCores.

Problem: B=4, S=2048, D=1024, H=16, Dh=64 MHA with key-side boolean mask.

Sharding: core c handles (batch b = c//2, head-half g = c%2, 8 heads each).
QKV are column-parallel, the output projection is row-parallel (Megatron
style); the host sums the two partial output projections per batch and adds
the output bias.

Host-side preprocessing (pure data marshalling, exact):
  - All inputs are pre-tiled into DMA-native layouts (partition-major,
    contiguous per partition) so each dma_start lowers to large linear
    descriptors instead of thousands of 2KB strided reads.
  - x is transposed per batch (the PE contracts over the partition dim, so
    x^T is required for every projection).
  - Keys with mask=False contribute exactly zero after softmax, so the host
    gathers only the unmasked keys (padded to a multiple of 384 with zero
    rows whose exp-bias is -1e30 => exp == 0 exactly). Exact, and cuts
    score/exp/attn-V work roughly in half.

On-core dataflow (all matmuls in float32r):
  xT --(Wk,Wv)--> KT[f,k], V[k,f] (+biases); KT rows 0:64 even head,
                  rows 64:128 odd head of each pair
  xT --(Wq)--> QT[f,q]
  scores^T[k,q]: per head PAIR, two row-tiled K=64 matmuls run
                 concurrently on disjoint PE row-groups (~the cost of one)
  E = exp(scores*0.125 + maskbias[k])      (one ScalarE pass, mask fused)
  out_aug[65,q] = [V_h | ones]^T x E       (row 64 = softmax denominator)
  rX[1,q] = 1/den (DVE exact reciprocal, off the PE critical path)
  rbX[64,q] = SBUF->SBUF DMA broadcast of rX (free-dim stride-0 read)
  attnT[f,q] = out_aug[0:64] * rbX         (DVE, one PSUM operand)
  out[s,D] = attnT^T x Wo                  (partial; host adds pair + bo)
"""

import os
import numpy as np

os.environ.setdefault("MYCRO_LOCAL_CACHE", "1")

D_MODEL = 1024
N_HEADS = 16
D_HEAD = 64
BATCH = 4
SEQ = 2048
N_CORES = 8
FH = 512          # features per core (8 heads x 64)
HPC = 8           # heads per core
NEG = -1.0e30     # additive bias for padded/masked keys; exp -> 0 exactly
# bf16 Schraudolph fast-exp constants (odd heads run exp on the DVE as one
# affine op + float->int16 convert; the int16 bits ARE the bf16 exp value)
SCH_A = 2.0 ** 7 / float(np.log(2.0))       # 2^7/ln2
SCH_B = 127.0 * 2.0 ** 7 - 5.59             # exponent bias - tuning const
SCH_PAD = -30.0                             # pad-key bias: exp(-30) ~ 1e-13

_COMPILED = {}    # k_pad -> nc
last_results = None  # BassKernelResults of the most recent run (for test.py)


def _build(k_pad):
    """Emit + compile the per-core bass kernel for a given padded key count."""
    import concourse.bacc as bacc
    import concourse.bass as bass
    import concourse.tile as tile
    from concourse import library_config, mybir

    f32 = mybir.dt.float32
    f32r = mybir.dt.float32r
    bf16 = mybir.dt.bfloat16
    i16 = mybir.dt.int16
    KT_N = k_pad // 128                     # number of 128-key tiles
    KC = 512 if k_pad % 512 == 0 else 384   # key-side chunk (fp32r needs N>=256)
    assert k_pad % KC == 0 and KC % 128 == 0
    NKC = k_pad // KC

    nc = bacc.Bacc("TRN2", target_bir_lowering=False, debug=False,
                   num_devices=N_CORES)

    # all pre-tiled on host into DMA-native layouts
    dxq = nc.dram_tensor("xq", [4, 128, 8, 512], f32r, kind="ExternalInput")
    dxk = nc.dram_tensor("xk", [NKC, 128, 8, KC], f32r, kind="ExternalInput")
    dWq = nc.dram_tensor("Wq", [128, 8, FH], f32r, kind="ExternalInput")
    dWk = nc.dram_tensor("Wk", [128, 8, FH], f32r, kind="ExternalInput")
    dWv = nc.dram_tensor("Wv", [128, 8, HPC * 65], f32r, kind="ExternalInput")
    dWo = nc.dram_tensor("Wo", [128, 4, D_MODEL], f32r, kind="ExternalInput")
    dbc = nc.dram_tensor("bcst", [128, 8 + 2 * KT_N], f32, kind="ExternalInput")
    dbv = nc.dram_tensor("bv", [HPC * 65], f32r, kind="ExternalInput")
    dcst = nc.dram_tensor("consts", [256], f32r, kind="ExternalInput")  # ones
    dout = nc.dram_tensor("out", [SEQ, D_MODEL], f32, kind="ExternalOutput")

    EXP = mybir.ActivationFunctionType.Exp
    IDn = mybir.ActivationFunctionType.Identity

    with tile.TileContext(nc) as tc:
        nc.gpsimd.load_library(library_config.attn)
        with tc.tile_pool(name="persist", bufs=1) as pers:
            # ---- constants in SBUF ----
            bc = pers.tile([128, 8 + 2 * KT_N], f32, tag="bcst")
            nc.sync.dma_start(out=bc, in_=dbc.ap())
            bq = bc[:, 0:4]
            bk = bc[:, 4:8]
            mb = bc[:, 8:8 + KT_N]
            mbx = bc[:, 8 + KT_N:8 + 2 * KT_N]    # Schraudolph bias per tile
            bv_row = pers.tile([1, HPC * 65], f32r, tag="bvr")
            nc.sync.dma_start(out=bv_row, in_=dbv.ap()[None, :])
            ones_t = pers.tile([1, 128], f32r, tag="ones")
            nc.sync.dma_start(out=ones_t, in_=dcst.ap()[None, 0:128])
            ones128 = ones_t[:, :]

            # ---- persistent activations ----
            QT = pers.tile([128, 4, SEQ], f32r, tag="QT")        # [f, q]
            # KT rows 0:64 = even head of pair, rows 64:128 = odd head.
            KT = pers.tile([128, 4, k_pad], f32r, tag="KT")
            Vau = pers.tile([128, KT_N, HPC, 65], bf16, tag="Vau")

            # ================= projections =================
            wtq_cm = tc.tile_pool(name="wtq", bufs=1)
            wtq = wtq_cm.__enter__()
            ppool_cm = tc.tile_pool(name="pp", bufs=4, space="PSUM")
            ppool = ppool_cm.__enter__()

            # ----- K side (KT, V) -----
            with tc.tile_pool(name="wtk", bufs=1) as wtk, \
                 tc.tile_pool(name="xk", bufs=1) as xkp:
                pk = ppool
                wk = wtk.tile([128, 8, FH], f32r, tag="wk")
                nc.sync.dma_start(out=wk, in_=dWk.ap())
                # load every xk chunk up front (no pool-rotation waits) and
                # order DMAs so the first matmuls are not stuck behind
                # weight traffic they do not need yet.
                xk_ts = []
                for kc in range(NKC):
                    xk_t = xkp.tile([128, 8, KC], f32r, tag=f"xk{kc}")
                    nc.sync.dma_start(out=xk_t, in_=dxk.ap()[kc])
                    xk_ts.append(xk_t)
                wv = wtk.tile([128, 8, HPC * 65], f32r, tag="wv")
                nc.sync.dma_start(out=wv, in_=dWv.ap())
                wq = wtq.tile([128, 8, FH], f32r, tag="wq")
                nc.sync.dma_start(out=wq, in_=dWq.ap())
                for kc in range(NKC):
                    xk_t = xk_ts[kc]
                    for ft in range(4):
                        ps = pk.tile([128, KC], f32, tag="pk")
                        for dt in range(8):
                            nc.tensor.matmul(
                                ps,
                                lhsT=wk[:, dt, ft * 128:(ft + 1) * 128],
                                rhs=xk_t[:, dt, :],
                                start=(dt == 0), stop=(dt == 7))
                        ks = slice(kc * KC, (kc + 1) * KC)
                        nc.scalar.activation(KT[:, ft, ks], ps, IDn,
                                             bias=bk[:, ft:ft + 1])
                    for kb in range(KC // 128):
                        kg = kc * (KC // 128) + kb
                        ps = pk.tile([128, HPC * 65], f32, tag="pk")
                        for dt in range(8):
                            nc.tensor.matmul(
                                ps[:, 0:512],
                                lhsT=xk_t[:, dt, kb * 128:(kb + 1) * 128],
                                rhs=wv[:, dt, 0:512],
                                start=(dt == 0), stop=False)
                            nc.tensor.matmul(
                                ps[:, 512:520],
                                lhsT=xk_t[:, dt, kb * 128:(kb + 1) * 128],
                                rhs=wv[:, dt, 512:520],
                                start=(dt == 0), stop=False)
                        nc.tensor.matmul(ps[:, 0:512], lhsT=ones128,
                                         rhs=bv_row[:, 0:512],
                                         start=False, stop=True)
                        nc.tensor.matmul(ps[:, 512:520], lhsT=ones128,
                                         rhs=bv_row[:, 512:520],
                                         start=False, stop=True)
                        nc.scalar.copy(Vau[:, kg, :, :], ps)

            # ----- Q side (QT) -----
            with tc.tile_pool(name="xq", bufs=2) as xqp:
                pq = ppool
                for qc in range(4):
                    xq_t = xqp.tile([128, 8, 512], f32r, tag="xq")
                    nc.sync.dma_start(out=xq_t, in_=dxq.ap()[qc])
                    for ft in range(4):
                        ps = pq.tile([128, 512], f32, tag="pk")
                        for dt in range(8):
                            nc.tensor.matmul(
                                ps,
                                lhsT=wq[:, dt, ft * 128:(ft + 1) * 128],
                                rhs=xq_t[:, dt, :],
                                start=(dt == 0), stop=(dt == 7))
                        nc.scalar.activation(QT[:, ft, qc * 512:(qc + 1) * 512],
                                             ps, IDn, bias=bq[:, ft:ft + 1])

            ppool_cm.__exit__(None, None, None)
            wtq_cm.__exit__(None, None, None)

            # ================= attention core =================
            att2_cm = tc.tile_pool(name="att2", bufs=1)
            att2 = att2_cm.__enter__()
            attnT = att2.tile([128, 4, SEQ], f32r, tag="attnT")  # [f, q]
            wo = att2.tile([128, 4, D_MODEL], f32r, tag="wo")
            nc.sync.dma_start(out=wo, in_=dWo.ap())
            with tc.tile_pool(name="et", bufs=2) as etp, \
                 tc.tile_pool(name="dn", bufs=2) as dnp, \
                 tc.tile_pool(name="rb", bufs=2) as rbp, \
                 tc.tile_pool(name="sp", bufs=2, space="PSUM") as sp, \
                 tc.tile_pool(name="av", bufs=2, space="PSUM") as avp:
                for qh in range(2):         # query half (1024 queries)
                    q0 = qh * 1024
                    # all 8 softmax denominators of this query half
                    dden = dnp.tile([8, 1024], f32, tag="dden")
                    for t in range(4):      # head pair (heads 2t, 2t+1)
                        avA = avp.tile([65, 1024], f32, tag="av")
                        avB = avp.tile([65, 1024], f32, tag="av")
                        for kt in range(KT_N):
                            kts = slice(kt * 128, (kt + 1) * 128)
                            sA = sp.tile([128, 1024], f32, tag="s")
                            sB = sp.tile([128, 1024], f32, tag="s")
                            for h in range(2):
                                c0, c1 = q0 + h * 512, q0 + (h + 1) * 512
                                cs = slice(h * 512, (h + 1) * 512)
                                # row-tiled concurrent K=64 pair
                                nc.tensor.matmul(
                                    sA[:, cs],
                                    lhsT=KT[0:64, t, kts],
                                    rhs=QT[0:64, t, c0:c1],
                                    start=True, stop=True)
                                nc.tensor.matmul(
                                    sB[:, cs],
                                    lhsT=KT[64:128, t, kts],
                                    rhs=QT[64:128, t, c0:c1],
                                    start=True, stop=True)
                            # head A: exact exp on ScalarE; head B: one-op
                            # Schraudolph exp on the DVE (affine + int16
                            # convert; the int16 bits ARE the bf16 value).
                            eA = etp.tile([128, 1024], bf16, tag="etA")
                            nc.scalar.activation(eA, sA, EXP,
                                                 bias=mb[:, kt:kt + 1], scale=0.125)
                            eB = etp.tile([128, 1024], i16, tag="etB")
                            nc.vector.tensor_scalar(
                                eB, sB, float(SCH_A * 0.125),
                                mbx[:, kt:kt + 1],
                                mybir.AluOpType.mult, mybir.AluOpType.add)
                            eBf = eB.bitcast(bf16)
                            for h in range(2):
                                cs = slice(h * 512, (h + 1) * 512)
                                nc.tensor.matmul(
                                    avA[:, cs], lhsT=Vau[:, kt, 2 * t, :],
                                    rhs=eA[:, cs],
                                    start=(kt == 0), stop=(kt == KT_N - 1))
                                nc.tensor.matmul(
                                    avB[:, cs], lhsT=Vau[:, kt, 2 * t + 1, :],
                                    rhs=eBf[:, cs],
                                    start=(kt == 0), stop=(kt == KT_N - 1))
                        # drain PSUM quickly: unnormalized attn rows into
                        # attnT; denominator rows stage at partition 0 (DVE
                        # base-partition rule) then DMA-hop into dden row.
                        for hh, (av_t, prow) in enumerate(((avA, 0), (avB, 64))):
                            nc.scalar.copy(
                                attnT[prow:prow + 64, t, q0:q0 + 1024],
                                av_t[0:64, :])
                            dstg = rbp.tile([1, 1024], f32, tag="dstg")
                            nc.scalar.copy(dstg, av_t[64:65, :])
                            nc.sync.dma_start(
                                out=dden[2 * t + hh:2 * t + hh + 1, :], in_=dstg)
                    # one batched reciprocal for the whole query half, then
                    # broadcast each row via SBUF->SBUF DMA and scale in place.
                    drec = dnp.tile([8, 1024], f32, tag="drec")
                    nc.vector.reciprocal(drec, dden)
                    for t in range(4):
                        rb_t = rbp.tile([128, 1024], f32, tag="rb")
                        for hh in range(2):
                            rsg = rbp.tile([1, 1024], f32, tag="rsg")
                            nc.sync.dma_start(
                                out=rsg,
                                in_=drec[2 * t + hh:2 * t + hh + 1, :])
                            if hh == 0:
                                nc.gpsimd.partition_broadcast(
                                    rb_t[0:64, :], rsg, channels=64)
                            else:
                                rtmp = rbp.tile([64, 1024], f32, tag="rtmp")
                                nc.gpsimd.partition_broadcast(
                                    rtmp, rsg, channels=64)
                                nc.vector.tensor_copy(rb_t[64:128, :], rtmp)
                        for hh in range(2):
                            prow = hh * 64
                            sl = attnT[prow:prow + 64, t, q0:q0 + 1024]
                            nc.vector.tensor_mul(sl, sl,
                                                 rb_t[prow:prow + 64, :])

            # ================= output projection (partial) =================
            with tc.tile_pool(name="op", bufs=2, space="PSUM") as opp, \
                 tc.tile_pool(name="ot", bufs=3) as otp:
                for st in range(16):
                    ps = opp.tile([128, D_MODEL], f32, tag="op")
                    for ft in range(4):
                        for dh in range(2):
                            nc.tensor.matmul(
                                ps[:, dh * 512:(dh + 1) * 512],
                                lhsT=attnT[:, ft, st * 128:(st + 1) * 128],
                                rhs=wo[:, ft, dh * 512:(dh + 1) * 512],
                                start=(ft == 0), stop=(ft == 3))
                    ot = otp.tile([128, D_MODEL], f32, tag="ot")
                    nc.vector.tensor_copy(ot, ps)
                    nc.sync.dma_start(out=dout.ap()[st * 128:(st + 1) * 128, :], in_=ot)
            att2_cm.__exit__(None, None, None)

    nc.compile()
    return nc


def _get_compiled(k_pad):
    if k_pad not in _COMPILED:
        _COMPILED[k_pad] = _build(k_pad)
    return _COMPILED[k_pad]


def _tile_pf(a, p=128):
    """[P*t, f...] -> contiguous [p, t, f...] partition-major tiling."""
    t = a.shape[0] // p
    return np.ascontiguousarray(
        a.reshape(t, p, *a.shape[1:]).swapaxes(0, 1))


def _prep_core_inputs(x, attention_mask, Wq, bq, Wk, bk, Wv, bv, Wo):
    """Host-side shard prep. Returns (in_maps, k_pad)."""
    x = np.asarray(x, np.float32)
    mask = np.asarray(attention_mask, bool)
    idxs = [np.nonzero(mask[b])[0] for b in range(BATCH)]
    ke_max = max(1, max(len(i) for i in idxs))
    k_pad = 384 * ((ke_max + 383) // 384)
    if k_pad > SEQ:
        k_pad = SEQ
    KC = 512 if k_pad % 512 == 0 else 384
    NKC = k_pad // KC
    KT_N = k_pad // 128

    consts = np.zeros(256, np.float32)
    consts[0:128] = 1.0

    in_maps = []
    for b in range(BATCH):
        xT = x[b].T                                  # [D, S] view
        # xq: [qc, p, dt, 512]
        xq = np.ascontiguousarray(
            xT.reshape(8, 128, 4, 512).transpose(2, 1, 0, 3))
        idx = idxs[b]
        ke = len(idx)
        if ke > k_pad:
            idx = idx[:k_pad]
            ke = k_pad
        xkT = np.zeros((D_MODEL, k_pad), np.float32)
        xkT[:, :ke] = x[b][idx].T
        # xk: [kc, p, dt, KC]
        xk = np.ascontiguousarray(
            xkT.reshape(8, 128, NKC, KC).transpose(2, 1, 0, 3))
        maskb = np.zeros(k_pad, np.float32)
        maskb[ke:] = NEG
        mb_t = _tile_pf(maskb)                       # [128, KT_N]
        # Schraudolph per-partition addend: A*maskbias + B (finite pad bias)
        mbexp = np.full(k_pad, SCH_B, np.float32)
        mbexp[ke:] = SCH_A * SCH_PAD + SCH_B
        mbx_t = _tile_pf(mbexp)                      # [128, KT_N]
        KT_N = k_pad // 128
        for g in range(2):
            fs = slice(g * FH, (g + 1) * FH)
            # Wv/bv padded with a ones column per head: the V-projection
            # matmul then produces [V_h | ones] directly (col = 0*x + 1.0).
            Wv_aug = np.zeros((D_MODEL, HPC * 65), np.float32)
            bv_aug = np.zeros(HPC * 65, np.float32)
            for h in range(HPC):
                Wv_aug[:, h * 65:h * 65 + 64] = Wv[:, g * FH + h * 64:
                                                   g * FH + (h + 1) * 64]
                bv_aug[h * 65:h * 65 + 64] = bv[g * FH + h * 64:
                                                g * FH + (h + 1) * 64]
                bv_aug[h * 65 + 64] = 1.0
            in_maps.append({
                "xq": xq,
                "xk": xk,
                "Wq": _tile_pf(np.asarray(Wq[:, fs], np.float32)),
                "Wk": _tile_pf(np.asarray(Wk[:, fs], np.float32)),
                "Wv": _tile_pf(Wv_aug),
                "Wo": _tile_pf(np.asarray(Wo[fs, :], np.float32)),
                "bcst": np.concatenate(
                    [_tile_pf(np.asarray(bq[fs], np.float32)),
                     _tile_pf(np.asarray(bk[fs], np.float32)),
                     mb_t, mbx_t], axis=1).astype(np.float32),
                "bv": bv_aug,
                "consts": consts,
            })
    return in_maps, k_pad


def kernel(x, attention_mask, Wq, bq, Wk, bk, Wv, bv, Wo, bo):
    global last_results
    from concourse.bass_utils import run_bass_kernel_spmd

    in_maps, k_pad = _prep_core_inputs(x, attention_mask, Wq, bq, Wk, bk, Wv, bv, Wo)
    nc = _get_compiled(k_pad)
    res = run_bass_kernel_spmd(nc, in_maps, core_ids=list(range(N_CORES)))
    last_results = res

    bo = np.asarray(bo, np.float32)
    out = np.empty((BATCH, SEQ, D_MODEL), np.float32)
    for b in range(BATCH):
        out[b] = res.results[2 * b]["out"] + res.results[2 * b + 1]["out"] + bo
    return out


# revision 28
# speedup vs baseline: 1.3183x; 1.0326x over previous
"""Multi-head attention kernel for 8 Trainium2 NeuronCores.

Problem: B=4, S=2048, D=1024, H=16, Dh=64 MHA with key-side boolean mask.

Sharding: core c handles (batch b = c//2, head-half g = c%2, 8 heads each).
QKV are column-parallel, the output projection is row-parallel (Megatron
style); the host sums the two partial output projections per batch and adds
the output bias.

Host-side preprocessing (pure data marshalling, exact):
  - All inputs are pre-tiled into DMA-native layouts (partition-major,
    contiguous per partition) so each dma_start lowers to large linear
    descriptors instead of thousands of 2KB strided reads.
  - x is transposed per batch (the PE contracts over the partition dim, so
    x^T is required for every projection).
  - Keys with mask=False contribute exactly zero after softmax, so the host
    gathers only the unmasked keys (padded to a multiple of 384 with zero
    rows whose exp-bias is -1e30 => exp == 0 exactly). Exact, and cuts
    score/exp/attn-V work roughly in half.

On-core dataflow (all matmuls in float32r):
  xT --(Wk,Wv)--> KT[f,k], V[k,f] (+biases); KT rows 0:64 even head,
                  rows 64:128 odd head of each pair
  xT --(Wq)--> QT[f,q]
  scores^T[k,q]: per head PAIR, two row-tiled K=64 matmuls run
                 concurrently on disjoint PE row-groups (~the cost of one)
  E = exp(scores*0.125 + maskbias[k])      (one ScalarE pass, mask fused)
  out_aug[65,q] = [V_h | ones]^T x E       (row 64 = softmax denominator)
  rX[1,q] = 1/den (DVE exact reciprocal, off the PE critical path)
  rbX[64,q] = SBUF->SBUF DMA broadcast of rX (free-dim stride-0 read)
  attnT[f,q] = out_aug[0:64] * rbX         (DVE, one PSUM operand)
  out[s,D] = attnT^T x Wo                  (partial; host adds pair + bo)
"""

import os
import numpy as np

os.environ.setdefault("MYCRO_LOCAL_CACHE", "1")

D_MODEL = 1024
N_HEADS = 16
D_HEAD = 64
BATCH = 4
SEQ = 2048
N_CORES = 8
FH = 512          # features per core (8 heads x 64)
HPC = 8           # heads per core
NEG = -1.0e30     # additive bias for padded/masked keys; exp -> 0 exactly
# bf16 Schraudolph fast-exp constants (odd heads run exp on the DVE as one
# affine op + float->int16 convert; the int16 bits ARE the bf16 exp value)
SCH_A = 2.0 ** 7 / float(np.log(2.0))       # 2^7/ln2
SCH_B = 127.0 * 2.0 ** 7 - 5.59             # exponent bias - tuning const
SCH_PAD = -30.0                             # pad-key bias: exp(-30) ~ 1e-13

_COMPILED = {}    # k_pad -> nc
last_results = None  # BassKernelResults of the most recent run (for test.py)


def _build(k_pad):
    """Emit + compile the per-core bass kernel for a given padded key count."""
    import concourse.bacc as bacc
    import concourse.bass as bass
    import concourse.tile as tile
    from concourse import library_config, mybir

    f32 = mybir.dt.float32
    f32r = mybir.dt.float32r
    bf16 = mybir.dt.bfloat16
    i16 = mybir.dt.int16
    KT_N = k_pad // 128                     # number of 128-key tiles
    KC = 512 if k_pad % 512 == 0 else 384   # key-side chunk (fp32r needs N>=256)
    assert k_pad % KC == 0 and KC % 128 == 0
    NKC = k_pad // KC

    nc = bacc.Bacc("TRN2", target_bir_lowering=False, debug=False,
                   num_devices=N_CORES)

    # all pre-tiled on host into DMA-native layouts
    dxq = nc.dram_tensor("xq", [4, 128, 8, 512], f32r, kind="ExternalInput")
    dxk = nc.dram_tensor("xk", [NKC, 128, 8, KC], f32r, kind="ExternalInput")
    dWq = nc.dram_tensor("Wq", [128, 8, FH], f32r, kind="ExternalInput")
    dWk = nc.dram_tensor("Wk", [128, 8, FH], f32r, kind="ExternalInput")
    dWv = nc.dram_tensor("Wv", [128, 8, HPC * 65], f32r, kind="ExternalInput")
    dWo = nc.dram_tensor("Wo", [128, 4, D_MODEL], f32r, kind="ExternalInput")
    dbc = nc.dram_tensor("bcst", [128, 8 + 2 * KT_N], f32, kind="ExternalInput")
    dbv = nc.dram_tensor("bv", [HPC * 65], f32r, kind="ExternalInput")
    dcst = nc.dram_tensor("consts", [256], f32r, kind="ExternalInput")  # ones
    dout = nc.dram_tensor("out", [SEQ, D_MODEL], f32, kind="ExternalOutput")

    EXP = mybir.ActivationFunctionType.Exp
    IDn = mybir.ActivationFunctionType.Identity

    with tile.TileContext(nc) as tc:
        nc.gpsimd.load_library(library_config.attn)
        with tc.tile_pool(name="persist", bufs=1) as pers:
            # ---- constants in SBUF ----
            bc = pers.tile([128, 8 + 2 * KT_N], f32, tag="bcst")
            nc.sync.dma_start(out=bc, in_=dbc.ap())
            bq = bc[:, 0:4]
            bk = bc[:, 4:8]
            mb = bc[:, 8:8 + KT_N]
            mbx = bc[:, 8 + KT_N:8 + 2 * KT_N]    # Schraudolph bias per tile
            bv_row = pers.tile([1, HPC * 65], f32r, tag="bvr")
            nc.sync.dma_start(out=bv_row, in_=dbv.ap()[None, :])
            ones_t = pers.tile([1, 128], f32r, tag="ones")
            nc.sync.dma_start(out=ones_t, in_=dcst.ap()[None, 0:128])
            ones128 = ones_t[:, :]

            # ---- persistent activations ----
            QT = pers.tile([128, 4, SEQ], f32r, tag="QT")        # [f, q]
            # KT rows 0:64 = even head of pair, rows 64:128 = odd head.
            KT = pers.tile([128, 4, k_pad], f32r, tag="KT")
            Vau = pers.tile([128, KT_N, HPC, 65], bf16, tag="Vau")

            # ================= projections =================
            wtq_cm = tc.tile_pool(name="wtq", bufs=1)
            wtq = wtq_cm.__enter__()
            ppool_cm = tc.tile_pool(name="pp", bufs=4, space="PSUM")
            ppool = ppool_cm.__enter__()

            # ----- K side (KT, V) -----
            with tc.tile_pool(name="wtk", bufs=1) as wtk, \
                 tc.tile_pool(name="xk", bufs=1) as xkp:
                pk = ppool
                wk = wtk.tile([128, 8, FH], f32r, tag="wk")
                nc.sync.dma_start(out=wk, in_=dWk.ap())
                # load every xk chunk up front (no pool-rotation waits) and
                # order DMAs so the first matmuls are not stuck behind
                # weight traffic they do not need yet.
                xk_ts = []
                for kc in range(NKC):
                    xk_t = xkp.tile([128, 8, KC], f32r, tag=f"xk{kc}")
                    nc.sync.dma_start(out=xk_t, in_=dxk.ap()[kc])
                    xk_ts.append(xk_t)
                wv = wtk.tile([128, 8, HPC * 65], f32r, tag="wv")
                nc.sync.dma_start(out=wv, in_=dWv.ap())
                wq = wtq.tile([128, 8, FH], f32r, tag="wq")
                nc.sync.dma_start(out=wq, in_=dWq.ap())
                for kc in range(NKC):
                    xk_t = xk_ts[kc]
                    for ft in range(4):
                        ps = pk.tile([128, KC], f32, tag="pk")
                        for dt in range(8):
                            nc.tensor.matmul(
                                ps,
                                lhsT=wk[:, dt, ft * 128:(ft + 1) * 128],
                                rhs=xk_t[:, dt, :],
                                start=(dt == 0), stop=(dt == 7))
                        ks = slice(kc * KC, (kc + 1) * KC)
                        nc.scalar.activation(KT[:, ft, ks], ps, IDn,
                                             bias=bk[:, ft:ft + 1])
                    for kb in range(KC // 128):
                        kg = kc * (KC // 128) + kb
                        ps = pk.tile([128, HPC * 65], f32, tag="pk")
                        for dt in range(8):
                            nc.tensor.matmul(
                                ps[:, 0:512],
                                lhsT=xk_t[:, dt, kb * 128:(kb + 1) * 128],
                                rhs=wv[:, dt, 0:512],
                                start=(dt == 0), stop=False)
                            nc.tensor.matmul(
                                ps[:, 512:520],
                                lhsT=xk_t[:, dt, kb * 128:(kb + 1) * 128],
                                rhs=wv[:, dt, 512:520],
                                start=(dt == 0), stop=False)
                        nc.tensor.matmul(ps[:, 0:512], lhsT=ones128,
                                         rhs=bv_row[:, 0:512],
                                         start=False, stop=True)
                        nc.tensor.matmul(ps[:, 512:520], lhsT=ones128,
                                         rhs=bv_row[:, 512:520],
                                         start=False, stop=True)
                        nc.scalar.copy(Vau[:, kg, :, :], ps)

            # ----- Q side (QT) -----
            with tc.tile_pool(name="xq", bufs=2) as xqp:
                pq = ppool
                for qc in range(4):
                    xq_t = xqp.tile([128, 8, 512], f32r, tag="xq")
                    nc.sync.dma_start(out=xq_t, in_=dxq.ap()[qc])
                    for ft in range(4):
                        ps = pq.tile([128, 512], f32, tag="pk")
                        for dt in range(8):
                            nc.tensor.matmul(
                                ps,
                                lhsT=wq[:, dt, ft * 128:(ft + 1) * 128],
                                rhs=xq_t[:, dt, :],
                                start=(dt == 0), stop=(dt == 7))
                        nc.scalar.activation(QT[:, ft, qc * 512:(qc + 1) * 512],
                                             ps, IDn, bias=bq[:, ft:ft + 1])

            ppool_cm.__exit__(None, None, None)
            wtq_cm.__exit__(None, None, None)

            # ================= attention core =================
            att2_cm = tc.tile_pool(name="att2", bufs=1)
            att2 = att2_cm.__enter__()
            attnT = att2.tile([128, 4, SEQ], f32r, tag="attnT")  # [f, q]
            wo = att2.tile([128, 4, D_MODEL], f32r, tag="wo")
            nc.sync.dma_start(out=wo, in_=dWo.ap())
            LN = mybir.ActivationFunctionType.Ln
            with tc.tile_pool(name="et", bufs=3) as etp, \
                 tc.tile_pool(name="rp", bufs=3) as rpp, \
                 tc.tile_pool(name="rb", bufs=3) as rbp, \
                 tc.tile_pool(name="sp", bufs=4, space="PSUM") as sp, \
                 tc.tile_pool(name="av", bufs=4, space="PSUM") as avp:
                for qc in range(4):         # 512-query chunk
                    q0 = qc * 512
                    qs = slice(q0, q0 + 512)
                    for t in range(4):      # head pair (heads 2t, 2t+1)
                        avA = avp.tile([65, 512], f32, tag="av")
                        avB = avp.tile([65, 512], f32, tag="av")
                        for kt in range(KT_N):
                            kts = slice(kt * 128, (kt + 1) * 128)
                            sA = sp.tile([128, 512], f32, tag="s")
                            sB = sp.tile([128, 512], f32, tag="s")
                            # row-tiled concurrent K=64 pair
                            nc.tensor.matmul(sA, lhsT=KT[0:64, t, kts],
                                             rhs=QT[0:64, t, qs],
                                             start=True, stop=True)
                            nc.tensor.matmul(sB, lhsT=KT[64:128, t, kts],
                                             rhs=QT[64:128, t, qs],
                                             start=True, stop=True)
                            # head A: exact exp on ScalarE; head B: one-op
                            # Schraudolph exp on the DVE (affine + int16
                            # convert; the int16 bits ARE the bf16 value).
                            eA = etp.tile([128, 512], bf16, tag="etA")
                            nc.scalar.activation(eA, sA, EXP,
                                                 bias=mb[:, kt:kt + 1],
                                                 scale=0.125)
                            eB = etp.tile([128, 512], i16, tag="etB")
                            nc.vector.tensor_scalar(
                                eB, sB, float(SCH_A * 0.125),
                                mbx[:, kt:kt + 1],
                                mybir.AluOpType.mult, mybir.AluOpType.add)
                            nc.tensor.matmul(
                                avA, lhsT=Vau[:, kt, 2 * t, :], rhs=eA,
                                start=(kt == 0), stop=(kt == KT_N - 1))
                            nc.tensor.matmul(
                                avB, lhsT=Vau[:, kt, 2 * t + 1, :],
                                rhs=eB.bitcast(bf16),
                                start=(kt == 0), stop=(kt == KT_N - 1))
                        # normalize: 1/den = exp(-ln(den)) on ScalarE (same
                        # act table), broadcast on GpSimd, one DVE multiply
                        # straight from PSUM into attnT.
                        for av_t, prow in ((avA, 0), (avB, 64)):
                            lt = rpp.tile([1, 512], f32, tag="lt")
                            nc.scalar.activation(lt, av_t[64:65, :], LN)
                            rt = rpp.tile([1, 512], f32, tag="rt")
                            nc.scalar.activation(rt, lt, EXP, scale=-1.0)
                            rb_t = rbp.tile([64, 512], f32, tag="rb")
                            nc.gpsimd.partition_broadcast(rb_t, rt, channels=64)
                            nc.vector.tensor_mul(
                                attnT[prow:prow + 64, t, qs],
                                av_t[0:64, :], rb_t)

            # ================= output projection (partial) =================
            with tc.tile_pool(name="op", bufs=2, space="PSUM") as opp, \
                 tc.tile_pool(name="ot", bufs=3) as otp:
                for st in range(16):
                    ps = opp.tile([128, D_MODEL], f32, tag="op")
                    for ft in range(4):
                        for dh in range(2):
                            nc.tensor.matmul(
                                ps[:, dh * 512:(dh + 1) * 512],
                                lhsT=attnT[:, ft, st * 128:(st + 1) * 128],
                                rhs=wo[:, ft, dh * 512:(dh + 1) * 512],
                                start=(ft == 0), stop=(ft == 3))
                    ot = otp.tile([128, D_MODEL], f32, tag="ot")
                    nc.vector.tensor_copy(ot, ps)
                    nc.sync.dma_start(out=dout.ap()[st * 128:(st + 1) * 128, :], in_=ot)
            att2_cm.__exit__(None, None, None)

    nc.compile()
    return nc


def _get_compiled(k_pad):
    if k_pad not in _COMPILED:
        _COMPILED[k_pad] = _build(k_pad)
    return _COMPILED[k_pad]


def _tile_pf(a, p=128):
    """[P*t, f...] -> contiguous [p, t, f...] partition-major tiling."""
    t = a.shape[0] // p
    return np.ascontiguousarray(
        a.reshape(t, p, *a.shape[1:]).swapaxes(0, 1))


def _prep_core_inputs(x, attention_mask, Wq, bq, Wk, bk, Wv, bv, Wo):
    """Host-side shard prep. Returns (in_maps, k_pad)."""
    x = np.asarray(x, np.float32)
    mask = np.asarray(attention_mask, bool)
    idxs = [np.nonzero(mask[b])[0] for b in range(BATCH)]
    ke_max = max(1, max(len(i) for i in idxs))
    k_pad = 384 * ((ke_max + 383) // 384)
    if k_pad > SEQ:
        k_pad = SEQ
    KC = 512 if k_pad % 512 == 0 else 384
    NKC = k_pad // KC
    KT_N = k_pad // 128

    consts = np.zeros(256, np.float32)
    consts[0:128] = 1.0

    in_maps = []
    for b in range(BATCH):
        xT = x[b].T                                  # [D, S] view
        # xq: [qc, p, dt, 512]
        xq = np.ascontiguousarray(
            xT.reshape(8, 128, 4, 512).transpose(2, 1, 0, 3))
        idx = idxs[b]
        ke = len(idx)
        if ke > k_pad:
            idx = idx[:k_pad]
            ke = k_pad
        xkT = np.zeros((D_MODEL, k_pad), np.float32)
        xkT[:, :ke] = x[b][idx].T
        # xk: [kc, p, dt, KC]
        xk = np.ascontiguousarray(
            xkT.reshape(8, 128, NKC, KC).transpose(2, 1, 0, 3))
        maskb = np.zeros(k_pad, np.float32)
        maskb[ke:] = NEG
        mb_t = _tile_pf(maskb)                       # [128, KT_N]
        # Schraudolph per-partition addend: A*maskbias + B (finite pad bias)
        mbexp = np.full(k_pad, SCH_B, np.float32)
        mbexp[ke:] = SCH_A * SCH_PAD + SCH_B
        mbx_t = _tile_pf(mbexp)                      # [128, KT_N]
        KT_N = k_pad // 128
        for g in range(2):
            fs = slice(g * FH, (g + 1) * FH)
            # Wv/bv padded with a ones column per head: the V-projection
            # matmul then produces [V_h | ones] directly (col = 0*x + 1.0).
            Wv_aug = np.zeros((D_MODEL, HPC * 65), np.float32)
            bv_aug = np.zeros(HPC * 65, np.float32)
            for h in range(HPC):
                Wv_aug[:, h * 65:h * 65 + 64] = Wv[:, g * FH + h * 64:
                                                   g * FH + (h + 1) * 64]
                bv_aug[h * 65:h * 65 + 64] = bv[g * FH + h * 64:
                                                g * FH + (h + 1) * 64]
                bv_aug[h * 65 + 64] = 1.0
            in_maps.append({
                "xq": xq,
                "xk": xk,
                "Wq": _tile_pf(np.asarray(Wq[:, fs], np.float32)),
                "Wk": _tile_pf(np.asarray(Wk[:, fs], np.float32)),
                "Wv": _tile_pf(Wv_aug),
                "Wo": _tile_pf(np.asarray(Wo[fs, :], np.float32)),
                "bcst": np.concatenate(
                    [_tile_pf(np.asarray(bq[fs], np.float32)),
                     _tile_pf(np.asarray(bk[fs], np.float32)),
                     mb_t, mbx_t], axis=1).astype(np.float32),
                "bv": bv_aug,
                "consts": consts,
            })
    return in_maps, k_pad


def kernel(x, attention_mask, Wq, bq, Wk, bk, Wv, bv, Wo, bo):
    global last_results
    from concourse.bass_utils import run_bass_kernel_spmd

    in_maps, k_pad = _prep_core_inputs(x, attention_mask, Wq, bq, Wk, bk, Wv, bv, Wo)
    nc = _get_compiled(k_pad)
    res = run_bass_kernel_spmd(nc, in_maps, core_ids=list(range(N_CORES)))
    last_results = res

    bo = np.asarray(bo, np.float32)
    out = np.empty((BATCH, SEQ, D_MODEL), np.float32)
    for b in range(BATCH):
        out[b] = res.results[2 * b]["out"] + res.results[2 * b + 1]["out"] + bo
    return out
